# revision 29
# baseline (speedup 1.0000x reference)
"""Neural CDE encoder for 8 Trainium2 NeuronCores — collective-free.

Math: all Euler times t_k = 0.05k (k=0..19) lie in spline interval 0, so
dX_k = base + mcoef_k * M1 with base = y1 - y0, M1 the knot-1 second
derivative (a fixed L-weighted sum of the trajectory), mcoef_k = t_k^2/2 - 1/6.
The vector-field contraction then collapses per batch item b to

    u_k = (Gb_b + mcoef_k * Gm_b) z_k,   Gb_b[h,h'] = sum_d W[hD+d,h'] base_b[d]

i.e. two precomputed (H,H) matrices per batch item. The Euler recurrence is
20 small per-b matvecs instead of 20 full (B,H)x(H,HD) matmuls: 8.6x fewer
FLOPs and — crucially — no cross-core state exchange.

Sharding: pure data parallel, 8 batch items per core, zero collectives.
Per core: stream the full W (as fp16, host-relayouted [d-halfstack, (hm,h')])
through the PE against a tiny stationary of base/M1 vectors to get G for its
8 items; bounce G through DRAM to transpose (X-bar DMA) into
[h'-partition, (b,which,h)] fp16 tiles; run the 20-step recurrence as per-b
PE matvecs (stationary = z^T fp16 column, scaled 1/64 for fp16 range; the
mcoef_k factor rides in a pre-scaled second stationary so Gb and Gm
accumulate in one psum group); interpolate the grid solution with J and
project with W_out on-device; host only concatenates per-core outputs.
"""

import numpy as np

B, L, D, H, O = 64, 128, 64, 768, 256
NS = 20
NC = 8
BL = B // NC          # 8 batch items per core
HM = H // 2           # 384 rows per stacked half
NV = HM * H           # 294912 V columns (hm, h')
GROWS = 2 * BL * H    # 12288 Graw rows (b, which, h)
ZSC = 1.0 / 64.0      # power-of-2 z scaling for fp16 stationary

_prog_cache = {}


def _host_constants():
    grid = (np.arange(NS + 1, dtype=np.float32) * np.float32(0.05)).astype(np.float32)
    grid[-1] = np.float32(1.0)
    dts = (grid[1:] - grid[:-1]).astype(np.float32)
    tk = grid[:-1].astype(np.float64)
    mcoef = (tk * tk / 2.0 - 1.0 / 6.0).astype(np.float32)

    # wg2 col0: base = y1 - y0; col1: M1 = <w, y>
    n = L - 2
    A = 4.0 * np.eye(n) + np.eye(n, k=1) + np.eye(n, k=-1)
    r0 = np.linalg.solve(A, np.eye(n)[:, 0])
    w = np.zeros(L, dtype=np.float64)
    w[0:n] += 6.0 * r0
    w[1:n + 1] += -12.0 * r0
    w[2:n + 2] += 6.0 * r0
    wg2 = np.zeros((L, 2), dtype=np.float32)
    wg2[0, 0] = -1.0
    wg2[1, 0] = 1.0
    wg2[:, 1] = w.astype(np.float32)

    # jgm[g, l]: interpolation weight of grid point g for output time l
    ts = np.linspace(0.0, 1.0, L, dtype=np.float32)
    j = np.clip(np.searchsorted(grid, ts, side="right") - 1, 0, NS - 1)
    wl = ((ts - grid[j]) / (grid[j + 1] - grid[j])).astype(np.float32)
    J = np.zeros((L, NS + 1), dtype=np.float32)
    J[np.arange(L), j] += 1.0 - wl
    J[np.arange(L), j + 1] += wl
    return dts, mcoef, wg2, np.ascontiguousarray(J.T)


def _build_program(dts, mcoef, has_blin, has_bout):
    import concourse.bacc as bacc
    import concourse.mybir as mybir
    import concourse.tile as tile

    f32 = mybir.dt.float32
    f32r = mybir.dt.float32r
    f16 = mybir.dt.float16
    ADD = mybir.AluOpType.add
    MUL = mybir.AluOpType.mult

    nc = bacc.Bacc("TRN2", target_bir_lowering=False, debug=False, num_devices=NC)

    trajl_d = nc.dram_tensor("trajl", [L, BL * D], f32r, kind="ExternalInput")
    t0aug_d = nc.dram_tensor("t0aug", [D + 1, BL], f32r, kind="ExternalInput")
    wz0_d = nc.dram_tensor("wz0", [D + 1, H], f32r, kind="ExternalInput")
    wg2_d = nc.dram_tensor("wg2", [L, 2], f32r, kind="ExternalInput")
    jgm_d = nc.dram_tensor("jgm", [NS + 1, L], f32r, kind="ExternalInput")
    wot_d = nc.dram_tensor("wot", [128, 6 * O], f32r, kind="ExternalInput")
    id4_d = nc.dram_tensor("ident4", [4, 4], f32r, kind="ExternalInput")
    id128_d = nc.dram_tensor("ident128", [128, 128], f32r, kind="ExternalInput")
    v_d = nc.dram_tensor("v", [2 * D, NV], f16, kind="ExternalInput")
    if has_blin:
        blv_d = nc.dram_tensor("blinv", [2 * D, HM], f16, kind="ExternalInput")
    if has_bout:
        bout_d = nc.dram_tensor("bout", [O, 1], f32, kind="ExternalInput")
    out_d = nc.dram_tensor("out", [BL, O, L], f32, kind="ExternalOutput")

    VT = 4096             # V dma tile cols
    NG = NV // VT         # 72 dma tiles
    NCH = NV // 512       # 576 matmul chunks

    with tile.TileContext(nc) as tc:
        with (
            tc.tile_pool(name="pers", bufs=1) as pers,
            tc.tile_pool(name="dram", bufs=1, space="DRAM") as dram,
        ):
            graw_d = dram.tile([GROWS, H], f16)

            sb_id4 = pers.tile([4, 4], f32r, tag="id4")
            nc.sync.dma_start(sb_id4[:], id4_d.ap())
            sb_id128 = pers.tile([128, 128], f32r, tag="id128")
            nc.sync.dma_start(sb_id128[:], id128_d.ap())
            sb_s2 = pers.tile([2 * D, 32], f16, tag="s2")
            # zT state, fp16, scaled by ZSC; columns are (g, j, bi)
            sb_zt = pers.tile([2 * D, 6 * BL], f16, tag="zt", name="sb_zt")
            sb_czt = pers.tile([2 * D, 6 * BL], f16, tag="czt", name="sb_czt")
            # column-layout z history: zh[p, k, (g, j, bi)] = z_k[4g+bi, 128j+p]
            sb_zh = pers.tile([128, NS + 1, 6 * BL], f32r, tag="zh", name="sb_zh")
            if has_blin:
                # bcol[p, w, (g, j, bi)] = sum_d blin[(128j+p)*D+d] * vec_w[b, d]
                sb_bcol = pers.tile([128, 2, 6 * BL], f32, tag="bcol")
            if has_bout:
                sb_bout = pers.tile([O // 2, 2], f32, tag="bout")
                nc.sync.dma_start(
                    sb_bout[:], bout_d.ap().rearrange("(oh o) x -> o (oh x)", oh=2))

            # ---- setup: base/M1 vectors, S2 stationary, z0 ----------------
            with (
                tc.tile_pool(name="setup", bufs=1) as sp,
                tc.tile_pool(name="pssu", bufs=3, space="PSUM") as pssu,
            ):
                sb_trajl = sp.tile([L, BL * D], f32r, tag="trajl")
                nc.sync.dma_start(sb_trajl[:], trajl_d.ap())
                sb_wg2 = sp.tile([L, 2], f32r, tag="wg2")
                nc.sync.dma_start(sb_wg2[:], wg2_d.ap())
                sb_t0 = sp.tile([D + 1, BL], f32r, tag="t0")
                nc.sync.dma_start(sb_t0[:], t0aug_d.ap())
                sb_wz0 = sp.tile([D + 1, H], f32r, tag="wz0")
                nc.sync.dma_start(sb_wz0[:], wz0_d.ap())

                ps_s = pssu.tile([D, 2 * BL], f32, tag="su", name="ps_s")
                for b in range(BL):
                    nc.tensor.matmul(ps_s[:, 2 * b:2 * b + 2],
                                     sb_trajl[:, D * b:D * (b + 1)],
                                     sb_wg2[:], start=True, stop=True)
                sb_ssb = sp.tile([D, 2 * BL], f16, tag="ssb")
                nc.vector.tensor_copy(sb_ssb[:], ps_s[:])
                nc.vector.memset(sb_s2[:], 0.0)
                s2v = sb_s2[:].rearrange("p (bw half) -> p bw half", half=2)
                nc.sync.dma_start(s2v[0:D, :, 0], sb_ssb[:])
                nc.sync.dma_start(s2v[D:2 * D, :, 1], sb_ssb[:])

                # z0 per group: row-form psum -> SBUF -> PE transpose into zh
                for g in range(2):
                    ps_z0 = pssu.tile([4, H], f32, tag="su", name="ps_z0")
                    for cs, ce in ((0, 512), (512, 768)):
                        nc.tensor.matmul(ps_z0[:, cs:ce],
                                         sb_t0[:, 4 * g:4 * (g + 1)],
                                         sb_wz0[:, cs:ce], start=True, stop=True)
                    z0sb = sp.tile([4, H], f32r, tag=f"z0sb{g}", name=f"z0sb{g}")
                    nc.vector.tensor_copy(z0sb[:], ps_z0[:])
                    ps_t0g = pssu.tile([128, 24], f32r, tag="su", name="ps_t0g")
                    for j in range(6):
                        nc.tensor.transpose(ps_t0g[:, 4 * j:4 * (j + 1)],
                                            z0sb[:, 128 * j:128 * (j + 1)],
                                            sb_id4[:])
                    nc.vector.tensor_copy(sb_zh[:, 0, 24 * g:24 * (g + 1)],
                                          ps_t0g[:])

            # ---- precompute G, bounce to DRAM -----------------------------
            with (
                tc.tile_pool(name="vp", bufs=2) as vp,
                tc.tile_pool(name="stp", bufs=3) as stp,
                tc.tile_pool(name="psg", bufs=2, space="PSUM") as psg,
            ):
                if has_blin:
                    sb_blv = vp.tile([2 * D, HM], f16, tag="blv", name="sb_blv")
                    nc.sync.dma_start(sb_blv[:], blv_d.ap())
                    ps_bv = psg.tile([32, HM], f32, tag="p4", name="ps_bv")
                    nc.tensor.matmul(ps_bv[:], sb_s2[:], sb_blv[:],
                                     start=True, stop=True)
                    sb_bsb = stp.tile([32, HM], f32r, tag="bsb", name="sb_bsb")
                    nc.vector.tensor_copy(sb_bsb[:], ps_bv[:])
                    # transpose [32, 128]-chunks; rows become hm-sub, cols (half,b,w)
                    bcv = sb_bcol[:].rearrange("p w (g j bi) -> p w g j bi",
                                               g=2, j=6, bi=4)
                    for t in range(3):
                        ps_bt = psg.tile([128, 32], f32r, tag="p4", name="ps_bt")
                        nc.tensor.transpose(ps_bt[:],
                                            sb_bsb[:, 128 * t:128 * (t + 1)],
                                            sb_id128[0:32, 0:32])
                        pbv = ps_bt[:].rearrange("p (b w half) -> p b w half",
                                                 b=BL, w=2, half=2)
                        for half in range(2):
                            for g in range(2):
                                nc.vector.tensor_copy(
                                    bcv[:, :, g, half * 3 + t, :],
                                    pbv[:, 4 * g:4 * (g + 1), :, half].rearrange(
                                        "p b w -> p w b"))

                # [32, 294912] view: row = (half, bw), col = flat (hm, h')
                graw_sc = graw_d[:].rearrange(
                    "(bw half hm) hp -> (bw half) (hm hp)",
                    bw=2 * BL, half=2, hm=HM)
                for gi in range(NG):
                    vt = vp.tile([2 * D, VT], f16, tag="vt", name="vt")
                    nc.sync.dma_start(vt[:], v_d.ap()[:, VT * gi:VT * (gi + 1)])
                    for h4 in range(2):
                        ps4 = psg.tile([128, 512], f32, tag="p4", name="ps4")
                        for q in range(4):
                            ch = 512 * (4 * h4 + q)
                            nc.tensor.matmul(ps4[32 * q:32 * (q + 1), :],
                                             sb_s2[:], vt[:, ch:ch + 512],
                                             start=True, stop=True,
                                             tile_position=(0, 32 * q))
                        stg = stp.tile([128, 512], f16, tag="stg", name="stg")
                        nc.vector.tensor_copy(stg[:], ps4[:])
                        for q in range(4):
                            c = 512 * (8 * gi + 4 * h4 + q)
                            nc.sync.dma_start(graw_sc[:, c:c + 512],
                                              stg[32 * q:32 * (q + 1), :])

            # ---- transpose G into [h'-partition, (b,which,h)] -------------
            sb_gt = []
            for j in range(6):
                gt = pers.tile([128, GROWS], f16, tag=f"gt{j}", name=f"gt{j}")
                nc.sync.dma_start(gt[:], graw_d[:][:, 128 * j:128 * (j + 1)],
                                  transpose=True)
                sb_gt.append(gt)

            # ---- recurrence ----------------------------------------------
            with (
                tc.tile_pool(name="ppu", bufs=2, space="PSUM") as ppu,
                tc.tile_pool(name="ppt", bufs=2, space="PSUM") as ppt,
                tc.tile_pool(name="ucpp", bufs=2) as ucpp,
            ):
                def refresh_zt(g, k):
                    # zT_b = z_k * ZSC (fp16), czT_b = mcoef[k] * zT_b
                    gs = slice(24 * g, 24 * (g + 1))
                    nc.vector.tensor_scalar_mul(
                        sb_zt[:, gs], sb_zh[:, k, gs], ZSC)
                    nc.vector.tensor_scalar_mul(
                        sb_czt[:, gs], sb_zh[:, k, gs], float(ZSC * mcoef[k]))

                def tail(g, k):
                    # zh[k+1] = zh[k] + dt/ZSC * u (u arrives row-wise in
                    # psum at partitions {0,32,64,96}; PE-transpose it back)
                    ucp = ucpp.tile([128, H], f32r, tag="ucp", name="ucp")
                    nc.vector.tensor_copy(ucp[:], ps_u[g][:])
                    pst = ppt.tile([128, 6 * 128], f32r, tag="pst", name="pst")
                    for hc in range(6):
                        nc.tensor.transpose(pst[:, 128 * hc:128 * (hc + 1)],
                                            ucp[:, 128 * hc:128 * (hc + 1)],
                                            sb_id128[:])
                    ut = pst[:].rearrange("p (hc q c) -> p c (hc q)", hc=6,
                                          q=4, c=32)[:, 0:1, :]
                    nc.vector.scalar_tensor_tensor(
                        sb_zh[:, k + 1:k + 2, 24 * g:24 * (g + 1)], ut,
                        float(dts[k] / ZSC),
                        sb_zh[:, k:k + 1, 24 * g:24 * (g + 1)],
                        op0=MUL, op1=ADD)
                    if has_blin:
                        for w in range(2):
                            sc = float(dts[k] * (1.0 if w == 0 else mcoef[k]))
                            nc.vector.scalar_tensor_tensor(
                                sb_zh[:, k + 1:k + 2, 24 * g:24 * (g + 1)],
                                sb_bcol[:, w:w + 1, 24 * g:24 * (g + 1)], sc,
                                sb_zh[:, k + 1:k + 2, 24 * g:24 * (g + 1)],
                                op0=MUL, op1=ADD)
                    if k + 1 < NS:
                        refresh_zt(g, k + 1)

                def mms(g):
                    ps = ppu.tile([128, H], f32, tag="P", name="ps_u")
                    for bi in range(4):
                        b = 4 * g + bi
                        for j in range(6):
                            for wh in range(2):
                                st = (sb_zt if wh == 0 else sb_czt)
                                stc = st[:, 24 * g + 4 * j + bi:
                                         24 * g + 4 * j + bi + 1]
                                off = (2 * b + wh) * H
                                first = (j == 0 and wh == 0)
                                last = (j == 5 and wh == 1)
                                nc.tensor.matmul(
                                    ps[32 * bi:32 * bi + 1, 0:512], stc,
                                    sb_gt[j][:, off:off + 512],
                                    start=first, stop=last,
                                    tile_position=(0, 32 * bi))
                                nc.tensor.matmul(
                                    ps[32 * bi:32 * bi + 1, 512:768], stc,
                                    sb_gt[j][:, off + 512:off + 768],
                                    start=first, stop=last,
                                    tile_position=(0, 32 * bi))
                    return ps

                ps_u = [None, None]
                refresh_zt(0, 0)
                refresh_zt(1, 0)
                ps_u[0] = mms(0)
                ps_u[1] = mms(1)
                for k in range(NS):
                    for g in range(2):
                        tail(g, k)
                        if k + 1 < NS:
                            ps_u[g] = mms(g)

            # ---- interpolate + project -----------------------------------
            with (
                tc.tile_pool(name="op", bufs=1) as op,
                tc.tile_pool(name="pso", bufs=4, space="PSUM") as pso,
                tc.tile_pool(name="osb", bufs=4) as osb,
            ):
                # z-grid into [grid-step partition, (b, h)] via PE transposes
                sb_zg = op.tile([NS + 1, BL * H], f32r, tag="zg")
                for b in range(BL):
                    g, bi = b // 4, b % 4
                    for j in range(6):
                        c = 24 * g + 4 * j + bi
                        ps_zc = pso.tile([NS + 1, 128], f32r, tag="pi",
                                         name="ps_zc")
                        nc.tensor.transpose(ps_zc[:], sb_zh[:, :, c],
                                            sb_id128[:])
                        nc.vector.tensor_copy(
                            sb_zg[:, H * b + 128 * j:H * b + 128 * (j + 1)],
                            ps_zc[:])
                sb_jg = op.tile([NS + 1, L], f32r, tag="jg")
                nc.sync.dma_start(sb_jg[:], jgm_d.ap())
                sb_wot = op.tile([128, 6 * O], f32r, tag="wot")
                nc.sync.dma_start(sb_wot[:], wot_d.ap())

                ztt = []
                for hs in range(6):
                    t = op.tile([128, BL * L], f32r, tag=f"ztt{hs}",
                                name=f"ztt{hs}")
                    ztt.append(t)
                for b in range(BL):
                    for hs in range(6):
                        ps_t = pso.tile([128, L], f32, tag="pi", name="ps_t")
                        nc.tensor.matmul(
                            ps_t[:], sb_zg[:, H * b + 128 * hs:H * b + 128 * (hs + 1)],
                            sb_jg[:], start=True, stop=True)
                        nc.vector.tensor_copy(ztt[hs][:, L * b:L * (b + 1)], ps_t[:])

                for oh in range(2):
                    for b in range(BL):
                        ps_o = pso.tile([128, L], f32, tag="po", name="ps_o")
                        for j in range(6):
                            oc = O * j + 128 * oh
                            nc.tensor.matmul(
                                ps_o[:], sb_wot[:, oc:oc + 128],
                                ztt[j][:, L * b:L * (b + 1)],
                                start=(j == 0), stop=(j == 5))
                        so = osb.tile([128, L], f32, tag="so", name="so")
                        if has_bout:
                            nc.vector.tensor_scalar(
                                so[:], ps_o[:], sb_bout[:, oh:oh + 1], None,
                                op0=ADD)
                        else:
                            nc.vector.tensor_copy(so[:], ps_o[:])
                        nc.sync.dma_start(
                            out_d.ap()[b, 128 * oh:128 * (oh + 1), :], so[:])

    nc.compile()
    return nc


def build_in_maps(inputs):
    traj = np.ascontiguousarray(np.asarray(inputs["traj"], dtype=np.float32))
    W_lin = np.asarray(inputs["W_lin"], dtype=np.float32)
    b_lin = np.asarray(inputs["b_lin"], dtype=np.float32)
    W_out = np.asarray(inputs["W_out"], dtype=np.float32)
    b_out = np.asarray(inputs["b_out"], dtype=np.float32)
    W_z0 = np.asarray(inputs["W_z0"], dtype=np.float32)
    b_z0 = np.asarray(inputs["b_z0"], dtype=np.float32)

    dts, mcoef, wg2, jgm = _host_constants()
    has_blin = bool(np.any(b_lin))
    has_bout = bool(np.any(b_out))

    Wr = W_lin.reshape(H, D, H)
    V = np.ascontiguousarray(np.concatenate(
        [Wr[:HM].transpose(1, 0, 2).reshape(D, NV),
         Wr[HM:].transpose(1, 0, 2).reshape(D, NV)], axis=0)).astype(np.float16)
    wz0_aug = np.ascontiguousarray(
        np.concatenate([W_z0.T, b_z0[None, :]], axis=0))
    wot = np.ascontiguousarray(
        W_out.T.reshape(6, 128, O).transpose(1, 0, 2).reshape(128, 6 * O))
    ident4 = np.eye(4, dtype=np.float32)
    ident128 = np.eye(128, dtype=np.float32)

    shared = dict(wz0=wz0_aug, wg2=wg2, jgm=jgm, wot=wot, ident4=ident4,
                  ident128=ident128, v=V)
    if has_blin:
        blr = b_lin.reshape(H, D)
        shared["blinv"] = np.ascontiguousarray(np.concatenate(
            [blr[:HM].T, blr[HM:].T], axis=0)).astype(np.float16)
    if has_bout:
        shared["bout"] = np.ascontiguousarray(b_out[:, None])

    in_maps = []
    for i in range(NC):
        bsl = slice(BL * i, BL * (i + 1))
        trajl = np.ascontiguousarray(
            traj[bsl].transpose(1, 0, 2).reshape(L, BL * D))
        t0aug = np.ascontiguousarray(np.concatenate(
            [traj[bsl, 0, :].T, np.ones((1, BL), np.float32)], axis=0))
        m = dict(trajl=trajl, t0aug=t0aug, **shared)
        in_maps.append(m)
    return in_maps, (has_blin, has_bout), (dts, mcoef)


def kernel(**inputs):
    from concourse.bass_utils import run_bass_kernel_spmd

    in_maps, key, (dts, mcoef) = build_in_maps(inputs)
    if key not in _prog_cache:
        _prog_cache[key] = _build_program(dts, mcoef, *key)
    nc = _prog_cache[key]

    res = run_bass_kernel_spmd(nc, in_maps, core_ids=list(range(NC)))
    out = np.concatenate([r["out"] for r in res.results], axis=0)
    return np.ascontiguousarray(out.transpose(0, 2, 1))
